# revision 6
# baseline (speedup 1.0000x reference)
"""GAT layer (2 steps) on 8 Trainium2 NeuronCores via Bass/Tile.

v2: descriptor-storm fixes + engine rebalance over the v1 baseline.
  - One-hot Q/QT tiles streamed as fp8 with partition-contiguous DRAM layout
    (one ~4KB descriptor per partition instead of 17x512B strided packets).
  - Mixed-dtype matmuls: fp8 one-hot lhsT x fp16 moving data.
  - Messages ex*h built per head with vector tensor_scalar (per-partition AP
    scalar) instead of 3D-broadcast multiplies + scalar broadcast copies.
  - H phase in bf16 (f32r eliminated), halves x/AllGather bytes.
  - AllGather split in two (blocks 0-4 / 5-9) and bf16, overlapping the tail
    of aggregation and the head of the next H phase.
  - Epilogue folds (1-alpha)/H into the reciprocal-denominator scale.
  - Edges sorted by src slot within each dst block; each gather reads only
    h_table[0:band] so early gathers can start while H phase still writes
    later blocks.
  - Gathers alternate SWDGE queues 0/1.
"""
import os
import sys

sys.path.insert(0, "/opt/trn_rl_repo")

import numpy as np

LAST_RES = None

N = 10000
E = 320000
F = 128
H = 4
HF = H * F  # 512
NDEV = 8
NPAD = 10240
NBLK = 80
NBLK_DEV = 10
DEVN = NBLK_DEV * 128  # 1280
ROWW = 640  # fp16 slots per table row (1280 B): h[0:512], el f32 @512:520, er f32 @520:528
GELEM = 640
SENT = NPAD  # sentinel row index for pad edges
C_SHIFT = 4.0
NEG_SLOPE = 0.2

_CACHE = {}


# ---------------------------------------------------------------- host prep
def _prep_graph(src, dst):
    # degree-balanced node -> slot permutation (equalizes per-block edge load)
    deg = np.bincount(dst, minlength=N)
    order_n = np.argsort(-deg, kind="stable")
    blk_load = np.zeros(NBLK, np.int64)
    blk_fill = np.zeros(NBLK, np.int64)
    slot = np.empty(N, np.int64)
    node_of_slot = np.full(NPAD, -1, np.int64)
    import heapq
    heap = [(0, 0, b) for b in range(NBLK)]
    heapq.heapify(heap)
    for n in order_n:
        while True:
            load, fill, b = heapq.heappop(heap)
            if fill < 128 and fill == blk_fill[b] and load == blk_load[b]:
                break
        s = 128 * b + fill
        slot[n] = s
        node_of_slot[s] = n
        blk_load[b] += deg[n]
        blk_fill[b] += 1
        if blk_fill[b] < 128:
            heapq.heappush(heap, (int(blk_load[b]), int(blk_fill[b]), b))
    free_slots = np.where(node_of_slot < 0)[0]
    for s, vn in zip(free_slots, range(N, N + len(free_slots))):
        node_of_slot[s] = vn
    sdst = slot[dst]
    ssrc = slot[src]
    # sort edges by (dst block, src slot): src-sorted within each block enables
    # banded gather dependencies on partially-written h_table
    blk_of = sdst // 128
    order = np.lexsort((ssrc, blk_of))
    s_src = ssrc[order]
    s_dst = sdst[order]
    blk = s_dst // 128
    counts = np.bincount(blk, minlength=NBLK)
    maxcnt = int(counts.max())
    nch = max(2, 2 * ((maxcnt + 255) // 256))  # even chunk count per block
    ebpad = nch * 128
    nhalf = nch // 2
    nipc = nhalf * 128  # idxs per gather call

    starts = np.zeros(NBLK + 1, np.int64)
    np.cumsum(counts, out=starts[1:])

    gidx = np.full((NBLK, ebpad), SENT, np.int64)
    dloc = np.zeros((NBLK, ebpad), np.int64)
    for b in range(NBLK):
        lo, hi = starts[b], starts[b + 1]
        cnt = hi - lo
        gidx[b, :cnt] = s_src[lo:hi]
        dloc[b, :cnt] = s_dst[lo:hi] - 128 * b

    import ml_dtypes
    f8 = ml_dtypes.float8_e4m3

    # round-robin ownership: device d owns global blocks d, 8+d, .., 32+d
    # (its "low" 5, AllGather half 0 = global blocks 0..39) and 40+d, 48+d, ..
    # (its "high" 5, half 1 = blocks 40..79). Step-1's H phase writes blocks
    # ascending, so the first half of the table only depends on collective 0.
    own_blocks = [[8 * j + d for j in range(5)] + [40 + 8 * j + d for j in range(5)]
                  for d in range(NDEV)]

    # per-call processed chunk count: trailing all-pad chunks are skipped
    # (their edges are sentinel pads contributing exactly zero)
    chmax = np.zeros(NBLK_DEV * 2, np.int64)
    for d in range(NDEV):
        for b in range(NBLK_DEV):
            cnt = int(counts[own_blocks[d][b]])
            chmax[2 * b] = max(chmax[2 * b], min(cnt, nipc))
            chmax[2 * b + 1] = max(chmax[2 * b + 1], max(cnt - nipc, 0))
    ch_call = tuple(int(x) for x in -(-chmax // 128))  # ceil/128 per call

    per_core = []
    bands = np.zeros((NDEV, NBLK_DEV * 2), np.int64)
    lead = np.zeros((NDEV, 2), np.int64)
    for d in range(NDEV):
        g = gidx[own_blocks[d]]  # [10, ebpad]
        dl = dloc[own_blocks[d]]

        # gather idx tiles: [10*2 calls, 128, nipc//16] int16
        bigidx = np.zeros((NBLK_DEV * 2, 128, nipc // 16), np.int16)
        gi = g.reshape(NBLK_DEV, 2, nipc)
        for b in range(NBLK_DEV):
            for hf_ in range(2):
                call = 2 * b + hf_
                v = gi[b, hf_]  # [nipc]
                t = v.reshape(nipc // 16, 16).T.astype(np.int16)
                bigidx[call] = np.tile(t, (8, 1))
                kept = v[:128 * ch_call[call]]
                bands[d, call] = int(kept.max()) + 1 if len(kept) else 1
                if call == 0:
                    lead[d, 0] = int(v[:384].max()) + 1
                    lead[d, 1] = int(v[:1152].max()) + 1

        # one-hot streams, partition-contiguous fp8:
        #   qcat[b][e, cc*128+n]  = (dloc[b, cc*128+e] == n)   (e on partitions)
        #   qtcat[b][n, cc*128+e] = (dloc[b, cc*128+e] == n)   (n on partitions)
        dl3 = dl.reshape(NBLK_DEV, nch, 128)
        qcat = np.zeros((NBLK_DEV, 128, nch * 128), f8)
        qtcat = np.zeros((NBLK_DEV, 128, nch * 128), f8)
        b_idx = np.repeat(np.arange(NBLK_DEV), nch * 128)
        cc_idx = np.tile(np.repeat(np.arange(nch), 128), NBLK_DEV)
        e_idx = np.tile(np.arange(128), NBLK_DEV * nch)
        n_idx = dl3.reshape(-1)
        one = f8(1.0)
        qcat[b_idx, e_idx, cc_idx * 128 + n_idx] = one
        qtcat[b_idx, n_idx, cc_idx * 128 + e_idx] = one

        per_core.append(dict(bigidx=bigidx, qcat=qcat, qtcat=qtcat))

    # SPMD: one compiled module for all devices -> band per call = max over devices
    band_call = bands.max(axis=0)  # [20]
    return (per_core, nch, slot, node_of_slot,
            tuple(int(x) for x in band_call), own_blocks, ch_call,
            tuple(int(x) for x in lead.max(axis=0)))


def _build(nch, alpha, band_call, ch_call, band_lead):
    import concourse.bass as bass
    import concourse.tile as tile
    from concourse import bacc, mybir

    f32 = mybir.dt.float32
    bf16 = mybir.dt.bfloat16
    f16 = mybir.dt.float16
    f8 = mybir.dt.float8e4
    i16 = mybir.dt.int16
    nhalf = nch // 2
    nipc = nhalf * 128
    icols = nipc // 16
    CA = float((1.0 - alpha) / H)

    nc = bacc.Bacc("TRN2", target_bir_lowering=False, debug=False,
                   num_devices=NDEV)

    # ---- params (shared across cores unless noted)
    xT0_p = nc.declare_dram_parameter("xT0g", [NBLK // 4, 128, 512], bf16, isOutput=False)
    W_p = nc.declare_dram_parameter("Wm", [128, HF], bf16, isOutput=False)
    ALR_p = nc.declare_dram_parameter("ALR", [128, 2 * H], bf16, isOutput=False)
    x0b_p = nc.declare_dram_parameter("x0b", [DEVN, F], f32, isOutput=False)  # per-core
    ident_p = nc.declare_dram_parameter("ident32", [128, 128], f32, isOutput=False)
    bigidx_p = nc.declare_dram_parameter("bigidx", [NBLK_DEV * 2, 128, icols], i16, isOutput=False)  # per-core
    er0_p = nc.declare_dram_parameter("er0", [128, NBLK_DEV, H], f16, isOutput=False)  # per-core
    qcat_p = nc.declare_dram_parameter("qcat", [NBLK_DEV, 128, nch * 128], f8, isOutput=False)  # per-core
    qtcat_p = nc.declare_dram_parameter("qtcat", [NBLK_DEV, 128, nch * 128], f8, isOutput=False)  # per-core
    sent_p = nc.declare_dram_parameter("sentrow", [1, ROWW], f16, isOutput=False)
    out_p = nc.declare_dram_parameter("outx", [DEVN, F], f32, isOutput=True)  # per-core

    # ---- internal DRAM
    h_table = nc.dram_tensor("h_table", [NPAD + 16, ROWW], f16)
    xt_own0 = nc.dram_tensor("xt_own0", [128, DEVN // 2], bf16)
    xt_own1 = nc.dram_tensor("xt_own1", [128, DEVN // 2], bf16)
    ag0 = nc.dram_tensor("ag0", [NDEV, 128, DEVN // 2], bf16, addr_space="Shared")
    ag1 = nc.dram_tensor("ag1", [NDEV, 128, DEVN // 2], bf16, addr_space="Shared")

    from contextlib import ExitStack
    with tile.TileContext(nc) as tc, ExitStack() as ctx:
        cpool = ctx.enter_context(tc.tile_pool(name="consts", bufs=1))
        gpool = ctx.enter_context(tc.tile_pool(name="gather", bufs=4))
        stpool = ctx.enter_context(tc.tile_pool(name="stage", bufs=4))
        xtpool = ctx.enter_context(tc.tile_pool(name="xt", bufs=3))
        qpool = ctx.enter_context(tc.tile_pool(name="q", bufs=2))
        mpool = ctx.enter_context(tc.tile_pool(name="msg", bufs=3))
        apool = ctx.enter_context(tc.tile_pool(name="attn", bufs=4))
        epool = ctx.enter_context(tc.tile_pool(name="epi", bufs=4))
        pbig = ctx.enter_context(tc.tile_pool(name="pbig", bufs=2, space="PSUM"))
        psm = ctx.enter_context(tc.tile_pool(name="psm", bufs=2, space="PSUM"))
        per = ctx.enter_context(tc.tile_pool(name="per", bufs=2, space="PSUM"))
        pden = ctx.enter_context(tc.tile_pool(name="pden", bufs=1, space="PSUM"))
        per2 = ctx.enter_context(tc.tile_pool(name="per2", bufs=1, space="PSUM"))

        # ---- load constants
        W_sb = cpool.tile([128, HF], bf16, tag="W")
        nc.sync.dma_start(out=W_sb[:], in_=W_p[:])
        ALR_sb = cpool.tile([128, 2 * H], bf16, tag="ALR")
        nc.sync.dma_start(out=ALR_sb[:], in_=ALR_p[:])
        bigidx_sb = cpool.tile([128, NBLK_DEV * 2 * icols], i16, tag="bigidx")
        nc.sync.dma_start(
            out=bigidx_sb[:].rearrange("p (k w) -> p k w", k=NBLK_DEV * 2),
            in_=bigidx_p.rearrange("k p w -> p k w"),
        )
        ident_sb = cpool.tile([128, 128], f32, tag="ident")
        nc.sync.dma_start(out=ident_sb[:], in_=ident_p[:])
        er16a_s0 = cpool.tile([128, NBLK_DEV, H], f16, tag="er_s0")
        nc.sync.dma_start(out=er16a_s0[:], in_=er0_p[:])
        er16a_s1 = cpool.tile([128, NBLK_DEV, H], f16, tag="er_s1")
        shift_sb = cpool.tile([128, 1], f32, tag="shift")
        nc.vector.memset(shift_sb[:], -C_SHIFT)
        sent_sb = cpool.tile([1, ROWW], f16, tag="sent")
        nc.sync.dma_start(out=sent_sb[:], in_=sent_p[:])
        nc.sync.dma_start(out=h_table[SENT:SENT + 1, :], in_=sent_sb[:])

        for step in range(2):
            # ================= H phase: h_ext table for all nodes (replicated)
            for c4 in range(NBLK // 4):
                xt4 = xtpool.tile([128, 4, 128], bf16, tag="xt")
                if step == 0:
                    nc.sync.dma_start(
                        out=xt4[:], in_=xT0_p[c4].rearrange("p (c q) -> p c q", c=4)
                    )
                else:
                    # round-robin ownership: blocks 4*c4..4*c4+3 live on ranks
                    # r0..r0+3 at the same slot; one 3D-AP load covers all four
                    c = 4 * c4
                    src_t = ag0 if c < 40 else ag1
                    cc_ = c if c < 40 else c - 40
                    r0, pos = cc_ % 8, cc_ // 8
                    nc.sync.dma_start(
                        out=xt4[:],
                        in_=src_t[r0:r0 + 4, :, 128 * pos:128 * (pos + 1)].rearrange(
                            "r p q -> p r q"),
                    )
                for j2 in range(2):
                    stage = stpool.tile([128, 2, ROWW], f16, tag="stage")
                    for jj in range(2):
                        c = 4 * c4 + 2 * j2 + jj
                        xt_sl = xt4[:, 2 * j2 + jj, :]
                        h_ps = pbig.tile([128, HF], f32, tag="big")
                        nc.tensor.matmul(out=h_ps[:], lhsT=xt_sl, rhs=W_sb[:],
                                         start=True, stop=True)
                        e_ps = psm.tile([128, 128], f32, tag="sm")
                        nc.tensor.matmul(
                            out=e_ps[:, 0:2 * H], lhsT=xt_sl, rhs=ALR_sb[:],
                            start=True, stop=True
                        )
                        nc.vector.tensor_copy(out=stage[:, jj, 0:HF], in_=h_ps[:])
                        nc.vector.tensor_copy(
                            out=stage[:, jj, HF:HF + 16].bitcast(f32), in_=e_ps[:, 0:2 * H]
                        )
                    c0 = 4 * c4 + 2 * j2
                    nc.sync.dma_start(
                        out=h_table[128 * c0:128 * (c0 + 2), :].rearrange(
                            "(j p) w -> p j w", p=128),
                        in_=stage[:],
                    )

            # ================= AGG phase: own blocks
            er16a = er16a_s0 if step == 0 else er16a_s1
            for b in range(NBLK_DEV):
                if step == 0 and b == 6:
                    # xt_own0 (blocks 0-4) is complete by now; firing here keeps
                    # the collective's input-wait off the critical gather stream
                    nc.gpsimd.collective_compute(
                        "AllGather",
                        bass.mybir.AluOpType.bypass,
                        replica_groups=[list(range(NDEV))],
                        ins=[xt_own0[:]],
                        outs=[ag0[:]],
                    )
                q_t = qpool.tile([128, nch, 128], f8, tag="q")
                nc.sync.dma_start(
                    out=q_t[:], in_=qcat_p[b].rearrange("p (c w) -> p c w", c=nch)
                )
                qt_t = qpool.tile([128, nch, 128], f8, tag="qt")
                nc.sync.dma_start(
                    out=qt_t[:], in_=qtcat_p[b].rearrange("p (c w) -> p c w", c=nch)
                )
                out_ps = pbig.tile([128, HF], f32, tag="big")
                den_ps = pden.tile([128, H], f32, tag="den")

                nche = ch_call[2 * b] + ch_call[2 * b + 1]
                seglist = []
                for hf_ in range(2):
                    call = 2 * b + hf_
                    ch = ch_call[call]
                    if b == 0 and hf_ == 0 and ch > 12:
                        # graduated lead: small first gathers with low table
                        # bands start the stream well before the H phase ends
                        seglist.append((hf_, 0, 3, band_lead[0]))
                        seglist.append((hf_, 3, 6, band_lead[1]))
                        seglist.append((hf_, 9, ch - 9, band_call[call]))
                    else:
                        seglist.append((hf_, 0, ch, band_call[call]))
                halves = []
                for hf_, c0, ch, band in seglist:
                    call = 2 * b + hf_
                    G = gpool.tile([128, nhalf, GELEM], f16, tag="G")
                    nc.gpsimd.dma_gather(
                        out_ap=G[:, 0:ch, :],
                        in_ap=h_table[0:band, 0:GELEM],
                        idxs_ap=bigidx_sb[:, call * icols + 8 * c0:call * icols + 8 * (c0 + ch)],
                        num_idxs=128 * ch,
                        num_idxs_reg=128 * ch,
                        elem_size=GELEM,
                        elem_step=ROWW,
                        single_packet=False,
                    )
                    er_ps = per.tile([128, 4 * nhalf], f32, tag="er")
                    for cc in range(ch):
                        cg = hf_ * nhalf + c0 + cc
                        nc.tensor.matmul(
                            out=er_ps[:, 4 * cc:4 * cc + 4],
                            lhsT=qt_t[:, cg, :],
                            rhs=er16a[:, b, :],
                            start=True, stop=True,
                        )
                    # batched attention math over the half-block
                    z = apool.tile([128, 4 * nhalf], f32, tag="z")
                    el_view = G[:, 0:ch, HF:HF + 8].bitcast(f32)  # [128, ch, 4]
                    nc.vector.tensor_tensor(
                        out=z[:, 0:4 * ch], in0=el_view, in1=er_ps[:, 0:4 * ch],
                        op=mybir.AluOpType.add
                    )
                    v = apool.tile([128, 4 * nhalf], f32, tag="v")
                    nc.vector.tensor_scalar(
                        out=v[:, 0:4 * ch], in0=z[:, 0:4 * ch], scalar1=NEG_SLOPE,
                        scalar2=None, op0=mybir.AluOpType.mult,
                    )
                    w = apool.tile([128, 4 * nhalf], f32, tag="w")
                    nc.vector.tensor_tensor(
                        out=w[:, 0:4 * ch], in0=z[:, 0:4 * ch], in1=v[:, 0:4 * ch],
                        op=mybir.AluOpType.max
                    )
                    ex16 = apool.tile([128, 4 * nhalf], f16, tag="ex")
                    nc.scalar.activation(
                        out=ex16[:, 0:4 * ch], in_=w[:, 0:4 * ch],
                        func=mybir.ActivationFunctionType.Exp,
                        bias=shift_sb[:, 0:1],
                    )
                    # all messages of the half in one broadcast multiply
                    bmsg = mpool.tile([128, nhalf, HF], f16, tag="bmsg")
                    nc.vector.tensor_tensor(
                        out=bmsg[:, 0:ch].rearrange("p c (h f) -> p c h f", h=H),
                        in0=G[:, 0:ch, 0:HF].rearrange("p c (h f) -> p c h f", h=H),
                        in1=ex16[:, 0:4 * ch].rearrange("p (c h) -> p c h", h=H)[:, :, :, None]
                            .to_broadcast([128, ch, H, F]),
                        op=mybir.AluOpType.mult,
                    )
                    halves.append((hf_, c0, ch, ex16, bmsg))

                ci = 0
                for hf_, c0, ch, ex16, bmsg in halves:
                    for cc in range(ch):
                        cg = hf_ * nhalf + c0 + cc
                        nc.tensor.matmul(
                            out=den_ps[:], lhsT=q_t[:, cg, :],
                            rhs=ex16[:, 4 * cc:4 * cc + 4],
                            start=(ci == 0), stop=(ci == nche - 1), skip_group_check=True,
                        )
                        nc.tensor.matmul(
                            out=out_ps[:], lhsT=q_t[:, cg, :],
                            rhs=bmsg[:, cc, :],
                            start=(ci == 0), stop=(ci == nche - 1), skip_group_check=True,
                        )
                        ci += 1

                # ---- epilogue for block b
                den_sb = epool.tile([128, H], f32, tag="den")
                nc.vector.tensor_scalar(
                    out=den_sb[:], in0=den_ps[:], scalar1=1e-30, scalar2=None,
                    op0=mybir.AluOpType.add,
                )
                rden = epool.tile([128, H], f32, tag="rden")
                nc.vector.reciprocal(out=rden[:], in_=den_sb[:])
                rdca = epool.tile([128, H], f32, tag="rdca")
                nc.vector.tensor_scalar(
                    out=rdca[:], in0=rden[:], scalar1=CA, scalar2=None,
                    op0=mybir.AluOpType.mult,
                )
                ms = []
                for hd in range(H):
                    m = epool.tile([128, F], f32, tag=f"m{hd}")
                    nc.scalar.activation(
                        out=m[:], in_=out_ps[:, F * hd:F * (hd + 1)],
                        func=mybir.ActivationFunctionType.Copy,
                        scale=rdca[:, hd:hd + 1],
                    )
                    ms.append(m)
                a01 = epool.tile([128, F], f32, tag="a01")
                nc.vector.tensor_tensor(out=a01[:], in0=ms[0][:], in1=ms[1][:], op=mybir.AluOpType.add)
                a23 = epool.tile([128, F], f32, tag="a23")
                nc.vector.tensor_tensor(out=a23[:], in0=ms[2][:], in1=ms[3][:], op=mybir.AluOpType.add)
                macc = epool.tile([128, F], f32, tag="macc")
                nc.vector.tensor_tensor(out=macc[:], in0=a01[:], in1=a23[:], op=mybir.AluOpType.add)
                x0b_t = epool.tile([128, F], f32, tag="x0b")
                nc.sync.dma_start(out=x0b_t[:], in_=x0b_p[128 * b:128 * (b + 1), :])
                outf = epool.tile([128, F], f32, tag="outf")
                nc.vector.tensor_tensor(out=outf[:], in0=macc[:], in1=x0b_t[:], op=mybir.AluOpType.add)
                if step == 0:
                    tp_ps = psm.tile([128, 128], f32, tag="sm")
                    nc.tensor.transpose(out=tp_ps[:], in_=outf[:], identity=ident_sb[:])
                    xtb = epool.tile([128, 128], bf16, tag="xtb")
                    nc.vector.tensor_copy(out=xtb[:], in_=tp_ps[:])
                    # next step's er for this (own) block: er1 = x1_own @ (W@attn_r)
                    er2_ps = per2.tile([128, H], f32, tag="er2")
                    nc.tensor.matmul(out=er2_ps[:], lhsT=xtb[:],
                                     rhs=ALR_sb[:, H:2 * H], start=True, stop=True)
                    nc.vector.tensor_copy(out=er16a_s1[:, b, :], in_=er2_ps[:])
                    if b < 5:
                        nc.sync.dma_start(out=xt_own0[:, 128 * b:128 * (b + 1)], in_=xtb[:])
                    else:
                        nc.sync.dma_start(out=xt_own1[:, 128 * (b - 5):128 * (b - 4)], in_=xtb[:])
                    if b == 9:
                        nc.gpsimd.collective_compute(
                            "AllGather",
                            bass.mybir.AluOpType.bypass,
                            replica_groups=[list(range(NDEV))],
                            ins=[xt_own1[:]],
                            outs=[ag1[:]],
                        )
                else:
                    nc.sync.dma_start(out=out_p[128 * b:128 * (b + 1), :], in_=outf[:])

    nc.compile()
    return nc


# ---------------------------------------------------------------- entry point
def kernel(x, x0, src, dst, alpha, W, attn_l, attn_r, bias):
    x = np.asarray(x, np.float32)
    x0 = np.asarray(x0, np.float32)
    src = np.asarray(src).astype(np.int64)
    dst = np.asarray(dst).astype(np.int64)
    alpha = float(np.asarray(alpha))
    W = np.asarray(W, np.float32)
    attn_l = np.asarray(attn_l, np.float32)
    attn_r = np.asarray(attn_r, np.float32)
    bias = np.asarray(bias, np.float32)

    (per_core, nch, slot, node_of_slot, band_call, own_blocks,
     ch_call, band_lead) = _prep_graph(src, dst)

    key = (nch, round(alpha, 9), band_call, ch_call, band_lead)
    if key not in _CACHE:
        _CACHE[key] = _build(nch, alpha, band_call, ch_call, band_lead)
    nc = _CACHE[key]

    import ml_dtypes
    bf = ml_dtypes.bfloat16

    # shared host inputs
    xpad = np.zeros((NPAD, F), np.float32)
    real = node_of_slot < N
    xpad[real] = x[node_of_slot[real]]
    xT0 = np.ascontiguousarray(xpad.T)  # [128, NPAD] slot-ordered
    xT0g = np.ascontiguousarray(
        xT0.reshape(128, NBLK // 4, 512).transpose(1, 0, 2)).astype(bf)
    ALR = np.zeros((128, 2 * H), np.float32)
    Wr = W.reshape(F, H, F)
    ALR[:, 0:H] = np.einsum("fhg,hg->fh", Wr, attn_l)
    ALR[:, H:2 * H] = np.einsum("fhg,hg->fh", Wr, attn_r)
    ident32 = np.eye(128, dtype=np.float32)
    bias_mean = bias.mean(axis=0)  # [F]
    x0b_full = np.zeros((NPAD, F), np.float32)
    x0b_full[real] = alpha * x0[node_of_slot[real]] + (1.0 - alpha) * bias_mean[None, :]
    sentrow = np.zeros((1, ROWW), np.float16)
    sv = sentrow.view(np.uint8)
    sv[0, 2 * HF:2 * HF + 16] = np.full(4, -1e30, np.float32).view(np.uint8)

    from concourse.bass_utils import run_bass_kernel_spmd

    er_full = xpad @ ALR[:, H:2 * H]  # [NPAD, H] f32
    in_maps = []
    for d in range(NDEV):
        pc = per_core[d]
        rows = np.concatenate([np.arange(128 * gb, 128 * (gb + 1))
                               for gb in own_blocks[d]])
        er0 = np.ascontiguousarray(
            er_full[rows].reshape(NBLK_DEV, 128, H)
            .transpose(1, 0, 2)).astype(np.float16)
        in_maps.append({
            "xT0g": xT0g, "Wm": W.astype(bf), "ALR": ALR.astype(bf),
            "x0b": x0b_full[rows],
            "ident32": ident32, "bigidx": pc["bigidx"],
            "er0": er0, "qcat": pc["qcat"], "qtcat": pc["qtcat"],
            "sentrow": sentrow,
        })
    global LAST_RES
    res = None
    for attempt in range(3):
        try:
            res = run_bass_kernel_spmd(
                nc, in_maps, list(range(NDEV)),
                trace=bool(os.environ.get("GAT_TRACE")),
            )
            break
        except Exception:
            if attempt == 2:
                raise
            import time as _time
            _time.sleep(2.0)
    LAST_RES = res
    out_slots = np.zeros((NPAD, F), np.float32)
    for d, r in enumerate(res.results):
        ox = r["outx"]
        for bl, gb in enumerate(own_blocks[d]):
            out_slots[128 * gb:128 * (gb + 1)] = ox[128 * bl:128 * (bl + 1)]
    return out_slots[slot[np.arange(N)]].astype(np.float32)


if __name__ == "__main__":
    rng = np.random.default_rng(0)
    x = rng.standard_normal((N, F), dtype=np.float32)
    x0 = rng.standard_normal((N, F), dtype=np.float32)
    src = rng.integers(0, N, E).astype(np.int32)
    dst = rng.integers(0, N, E).astype(np.int32)
    W = (rng.standard_normal((F, H * F)).astype(np.float32) / np.sqrt(F))
    al = (rng.standard_normal((H, F)).astype(np.float32) / np.sqrt(F))
    ar = (rng.standard_normal((H, F)).astype(np.float32) / np.sqrt(F))
    bias = np.zeros((H, F), np.float32)
    out = kernel(x=x, x0=x0, src=src, dst=dst, alpha=np.float32(0.1),
                 W=W, attn_l=al, attn_r=ar, bias=bias)
    print("out", out.shape, out.dtype, float(np.abs(out).max()))
